# revision 5
# baseline (speedup 1.0000x reference)
"""FP8 GEMM kernel for Trainium2 (8 NeuronCores, SPMD data-parallel over tokens).

Computes: out = fp16( fp32( e5m2(x) @ e4m3(weight.T) ) + bias )
  x      [4, 4096, 4096] fp16
  weight [4096, 4096]    fp16  (out_features, in_features)
  bias   [4096]          fp16
  out    [4, 4096, 4096] fp16

Sharding: token dim (B*S = 16384) split across 8 cores (2048 rows each);
weight + bias replicated. No collectives; host concatenates the outputs.

Layout: the host quantizes to fp8 (ml_dtypes RNE — bit-identical to the
reference's jnp casts for these value ranges) and pre-packs both operands
into per-tile K-major blocks (`[tile][ki=128][ko=32][free]`), so every
device load is a fully contiguous plain-copy DMA. fp8 bits ship as uint8
tensors (the PJRT path rejects fp8 dtypes) and are bitcast at DMA issue.

Per-core kernel:
 - Loads are plain HWDGE copies (no cast): weights stream on the scalar
   (ACT) ring, x/bias/stores on the sync (SP) ring. Half the HBM bytes of
   the fp16+cast path and ~2x the bandwidth, so delivery stays ahead of
   the PE from ~1us on (the old SWDGE cast path idled the PE for 29us).
 - DoubleRow fp8 matmuls (K=256/instr, moving free dim 2x512 at the
   ~213ns/MM streaming floor) accumulate fp32 into PSUM. All of x8 stays
   resident (64KB/part); w8 n-tiles stream through a 3-deep pool.
 - The first two n-tile columns are interleaved per m-tile so the ramp
   only needs one x tile per ~6.9us of PE work.
 - Bias add fused into the PSUM eviction on DVE (its only job); output
   stores + bias broadcast go out on the sync HWDGE queue.
"""

import sys

if "/opt/trn_rl_repo" not in sys.path:
    sys.path.insert(0, "/opt/trn_rl_repo")

import numpy as np

B, S, DIN, DOUT = 4, 4096, 4096, 4096
NCORES = 8
M_TOTAL = B * S              # 16384
M_LOC = M_TOTAL // NCORES    # 2048
P = 128
M_TILES = M_LOC // P         # 16 m-tiles of 128 rows
N_TILE = 512
N_TILES = DOUT // N_TILE     # 8
K_SUB = DIN // P             # 32 k-subtiles of 128
K_CHUNKS = K_SUB // 2        # 16 DoubleRow chunks of 256

_cached_nc = None


def _build():
    global _cached_nc
    if _cached_nc is not None:
        return _cached_nc

    import concourse.mybir as mybir
    import concourse.tile as tile
    from concourse import bacc

    nc = bacc.Bacc("TRN2", target_bir_lowering=False, debug=False,
                   num_devices=NCORES)

    # host-packed fp8 K-major tile blocks, shipped as uint8 bits
    xd = nc.dram_tensor("xd", [M_TILES, P, K_SUB, P], mybir.dt.uint8,
                        kind="ExternalInput")
    wd = nc.dram_tensor("wd", [N_TILES, P, K_SUB, N_TILE], mybir.dt.uint8,
                        kind="ExternalInput")
    bvec = nc.dram_tensor("bvec", [DOUT], mybir.dt.float16,
                          kind="ExternalInput")
    out = nc.dram_tensor("out", [M_LOC, DOUT], mybir.dt.float16,
                         kind="ExternalOutput")

    with tile.TileContext(nc) as tc:
        with tc.tile_pool(name="w8p", bufs=3) as w8p, \
             tc.tile_pool(name="x8p", bufs=1) as x8p, \
             tc.tile_pool(name="outp", bufs=8) as outp, \
             tc.tile_pool(name="cst", bufs=1) as cst, \
             tc.tile_pool(name="psum", bufs=8, space="PSUM") as psump:

            # resident fp8 x: 16 tiles of [ki, ko, 128] e5m2
            x8 = [x8p.tile([P, K_SUB, P], mybir.dt.float8e5,
                           tag=f"x8_{m}", name=f"x8_{m}")
                  for m in range(M_TILES)]

            w8 = {}

            def load_w(j, bounds=None, eng=None):
                # bounds: ko-split points so the first matmuls can start
                # before the whole tile lands
                w8[j] = w8p.tile([P, K_SUB, N_TILE], mybir.dt.float8e4,
                                 tag="w8", name=f"w8_{j}")
                if bounds is None:
                    bounds = [0, K_SUB]
                if eng is None:
                    eng = nc.scalar
                for a, b in zip(bounds[:-1], bounds[1:]):
                    ko = slice(a, b)
                    eng.dma_start(
                        w8[j][:, ko, :],
                        wd[j, :, ko, :].bitcast(mybir.dt.float8e4))

            def load_x(m):
                nc.sync.dma_start(
                    x8[m][:],
                    xd[m, :, :, :].bitcast(mybir.dt.float8e5))

            # prologue: w8[0] streams on the scalar ring (small first chunk
            # so the first matmul fires early) while x tiles + w8[1] stream
            # in parallel on the sync ring — phase 1 goes PE-bound from
            # ~18us instead of ~28us.
            load_x(0)
            load_w(0, bounds=[0, 2, 8, 14, 20, 26, 32])
            load_x(1)
            load_w(1, bounds=[0, 16, 32], eng=nc.sync)

            # bias replicated across the 128 partitions (HWDGE broadcast);
            # only needed at the first PSUM eviction, which has 8 banks of
            # slack, so it can trail the weight tiles
            bias_rep = cst.tile([P, DOUT], mybir.dt.float16)
            nc.sync.dma_start(bias_rep[:],
                              bvec.ap()[None, :].to_broadcast((P, DOUT)))

            for m in range(2, 6):
                load_x(m)

            def do_group(j, m):
                wtile = w8[j]
                ps = psump.tile([P, N_TILE], mybir.dt.float32, tag="ps",
                                name=f"ps_{j}_{m}")
                for kc in range(K_CHUNKS):
                    nc.tensor.matmul(
                        ps[:],
                        x8[m][:, 2 * kc:2 * kc + 2, :],
                        wtile[:, 2 * kc:2 * kc + 2, :],
                        start=(kc == 0),
                        stop=(kc == K_CHUNKS - 1),
                        perf_mode=mybir.MatmulPerfMode.DoubleRow,
                    )
                ob = outp.tile([P, N_TILE], mybir.dt.float16, tag="ob",
                               name=f"ob_{j}_{m}")
                nc.vector.tensor_add(
                    ob[:], ps[:], bias_rep[:, j * N_TILE:(j + 1) * N_TILE])
                nc.sync.dma_start(
                    out[m * P:(m + 1) * P,
                        j * N_TILE:(j + 1) * N_TILE], ob[:])

            # ---- phase 1: columns 0+1 interleaved per m-tile, so the ramp
            # only needs one new x tile per two psum groups ----
            for m in range(M_TILES):
                if m + 6 < M_TILES:
                    load_x(m + 6)
                if m == 8:
                    load_w(2)
                do_group(0, m)
                do_group(1, m)

            # ---- phase 2: remaining columns, m-inner ----
            for j in range(2, N_TILES):
                for m in range(M_TILES):
                    if m == 0 and j + 1 < N_TILES:
                        load_w(j + 1, bounds=[0, 16, 32])
                    do_group(j, m)

    nc.compile()
    _cached_nc = nc
    return nc


def make_in_maps(x, weight, bias):
    import ml_dtypes

    x = np.asarray(x)
    weight = np.asarray(weight)
    bias = np.ascontiguousarray(np.asarray(bias))
    assert x.dtype == np.float16 and weight.dtype == np.float16

    # host-side fp8 quantization (RNE, matches the reference's jnp casts:
    # |x| << e5m2 max, |w| << 240 so OCP e4m3fn bits == TRN e4m3 values)
    x8 = x.astype(ml_dtypes.float8_e5m2).view(np.uint8)
    w8 = weight.astype(ml_dtypes.float8_e4m3fn).view(np.uint8)

    # weight [DOUT, DIN] -> [j, ki, ko, n]: wd[j,ki,ko,n] = w8[j*512+n,
    # ko*128+ki] (i.e. weight.T in per-tile K-major blocks)
    wd = np.ascontiguousarray(
        w8.reshape(N_TILES, N_TILE, K_SUB, P).transpose(0, 3, 2, 1))

    xf = x8.reshape(M_TOTAL, DIN)
    in_maps = []
    for c in range(NCORES):
        xc = xf[c * M_LOC:(c + 1) * M_LOC]
        # [M_LOC, DIN] -> [m-tile, ki, ko, m]: xd[t,ki,ko,m] = xc[t*128+m,
        # ko*128+ki]
        xd = np.ascontiguousarray(
            xc.reshape(M_TILES, P, K_SUB, P).transpose(0, 3, 2, 1))
        in_maps.append({"xd": xd, "wd": wd, "bvec": bias})
    return in_maps


def gather_out(results):
    out = np.concatenate([r["out"] for r in results], axis=0)
    return out.reshape(B, S, DOUT)


def kernel(x, weight, bias):
    from concourse.bass_utils import run_bass_kernel_spmd

    nc = _build()
    in_maps = make_in_maps(x, weight, bias)
    res = run_bass_kernel_spmd(nc, in_maps, core_ids=list(range(NCORES)))
    return gather_out(res.results)


# revision 8
# speedup vs baseline: 1.0095x; 1.0095x over previous
"""FP8 GEMM kernel for Trainium2 (8 NeuronCores, SPMD data-parallel over tokens).

Computes: out = fp16( fp32( e5m2(x) @ e4m3(weight.T) ) + bias )
  x      [4, 4096, 4096] fp16
  weight [4096, 4096]    fp16  (out_features, in_features)
  bias   [4096]          fp16
  out    [4, 4096, 4096] fp16

Sharding: token dim (B*S = 16384) split across 8 cores (2048 rows each);
weight + bias replicated. No collectives; host concatenates the outputs.

Layout: the host quantizes to fp8 (ml_dtypes RNE — bit-identical to the
reference's jnp casts for these value ranges) and pre-packs both operands
into per-tile K-major blocks (`[tile][ki=128][ko=32][free]`), so every
device load is a fully contiguous plain-copy DMA. fp8 bits ship as uint8
tensors (the PJRT path rejects fp8 dtypes) and are bitcast at DMA issue.

Per-core kernel:
 - Loads are plain HWDGE copies (no cast): weights stream on the scalar
   (ACT) ring, x/bias/stores on the sync (SP) ring. Half the HBM bytes of
   the fp16+cast path and ~2x the bandwidth, so delivery stays ahead of
   the PE from ~1us on (the old SWDGE cast path idled the PE for 29us).
 - DoubleRow fp8 matmuls (K=256/instr, moving free dim 2x512 at the
   ~213ns/MM streaming floor) accumulate fp32 into PSUM. All of x8 stays
   resident (64KB/part); w8 n-tiles stream through a 3-deep pool.
 - The first two n-tile columns are interleaved per m-tile so the ramp
   only needs one x tile per ~6.9us of PE work.
 - Bias add fused into the PSUM eviction on DVE (its only job); output
   stores + bias broadcast go out on the sync HWDGE queue.
"""

import sys

if "/opt/trn_rl_repo" not in sys.path:
    sys.path.insert(0, "/opt/trn_rl_repo")

import numpy as np

B, S, DIN, DOUT = 4, 4096, 4096, 4096
NCORES = 8
M_TOTAL = B * S              # 16384
M_LOC = M_TOTAL // NCORES    # 2048
P = 128
M_TILES = M_LOC // P         # 16 m-tiles of 128 rows
N_TILE = 512
N_TILES = DOUT // N_TILE     # 8
K_SUB = DIN // P             # 32 k-subtiles of 128
K_CHUNKS = K_SUB // 2        # 16 DoubleRow chunks of 256

_cached_nc = None


def _build():
    global _cached_nc
    if _cached_nc is not None:
        return _cached_nc

    import concourse.mybir as mybir
    import concourse.tile as tile
    from concourse import bacc

    nc = bacc.Bacc("TRN2", target_bir_lowering=False, debug=False,
                   num_devices=NCORES)

    # host-packed fp8 K-major tile blocks, shipped as uint8 bits
    xd = nc.dram_tensor("xd", [M_TILES, P, K_SUB, P], mybir.dt.uint8,
                        kind="ExternalInput")
    wd = nc.dram_tensor("wd", [N_TILES, P, K_SUB, N_TILE], mybir.dt.uint8,
                        kind="ExternalInput")
    bvec = nc.dram_tensor("bvec", [DOUT], mybir.dt.float16,
                          kind="ExternalInput")
    out = nc.dram_tensor("out", [M_LOC, DOUT], mybir.dt.float16,
                         kind="ExternalOutput")

    with tile.TileContext(nc) as tc:
        with tc.tile_pool(name="w8p", bufs=3) as w8p, \
             tc.tile_pool(name="x8p", bufs=1) as x8p, \
             tc.tile_pool(name="outp", bufs=8) as outp, \
             tc.tile_pool(name="cst", bufs=1) as cst, \
             tc.tile_pool(name="psum", bufs=8, space="PSUM") as psump:

            # resident fp8 x: 16 tiles of [ki, ko, 128] e5m2
            x8 = [x8p.tile([P, K_SUB, P], mybir.dt.float8e5,
                           tag=f"x8_{m}", name=f"x8_{m}")
                  for m in range(M_TILES)]

            w8 = {}

            def load_w(j, bounds=None, eng=None):
                # bounds: ko-split points so the first matmuls can start
                # before the whole tile lands
                w8[j] = w8p.tile([P, K_SUB, N_TILE], mybir.dt.float8e4,
                                 tag="w8", name=f"w8_{j}")
                if bounds is None:
                    bounds = [0, K_SUB]
                if eng is None:
                    eng = nc.scalar
                for a, b in zip(bounds[:-1], bounds[1:]):
                    ko = slice(a, b)
                    eng.dma_start(
                        w8[j][:, ko, :],
                        wd[j, :, ko, :].bitcast(mybir.dt.float8e4))

            def load_x(m):
                nc.sync.dma_start(
                    x8[m][:],
                    xd[m, :, :, :].bitcast(mybir.dt.float8e5))

            # prologue: only x8[0] + w8[0] gate the first matmuls, so they
            # get both HWDGE rings to themselves (w0 chunks alternate
            # rings); everything else (x1-5, w1, bias) queues behind.
            # All-8-cores-simultaneous HBM pull runs ~250GB/s per core, so
            # w0 lands ~15us in and pass 1 (j=0 only) is PE-bound after.
            load_x(0)
            w8[0] = w8p.tile([P, K_SUB, N_TILE], mybir.dt.float8e4,
                             tag="w8", name="w8_0")
            w0_bounds = [0, 2, 8, 14, 20, 26, 32]
            for i, (a, b) in enumerate(zip(w0_bounds[:-1], w0_bounds[1:])):
                eng = nc.scalar if i % 2 == 0 else nc.sync
                eng.dma_start(w8[0][:, a:b, :],
                              wd[0, :, a:b, :].bitcast(mybir.dt.float8e4))
            load_x(1)
            load_w(1, bounds=[0, 8, 16, 24, 32])

            # bias replicated across the 128 partitions (HWDGE broadcast);
            # only needed at the first PSUM eviction, which has 8 banks of
            # slack, so it can trail the weight tiles
            bias_rep = cst.tile([P, DOUT], mybir.dt.float16)
            nc.sync.dma_start(bias_rep[:],
                              bvec.ap()[None, :].to_broadcast((P, DOUT)))

            for m in range(2, 6):
                load_x(m)

            def do_group(j, m):
                wtile = w8[j]
                ps = psump.tile([P, N_TILE], mybir.dt.float32, tag="ps",
                                name=f"ps_{j}_{m}")
                for kc in range(K_CHUNKS):
                    nc.tensor.matmul(
                        ps[:],
                        x8[m][:, 2 * kc:2 * kc + 2, :],
                        wtile[:, 2 * kc:2 * kc + 2, :],
                        start=(kc == 0),
                        stop=(kc == K_CHUNKS - 1),
                        perf_mode=mybir.MatmulPerfMode.DoubleRow,
                    )
                ob = outp.tile([P, N_TILE], mybir.dt.float16, tag="ob",
                               name=f"ob_{j}_{m}")
                nc.vector.tensor_add(
                    ob[:], ps[:], bias_rep[:, j * N_TILE:(j + 1) * N_TILE])
                nc.sync.dma_start(
                    out[m * P:(m + 1) * P,
                        j * N_TILE:(j + 1) * N_TILE], ob[:])

            # ---- pass 1: column 0 alone — its only upfront needs are x0
            # and w0; x tiles + later w tiles stream in behind ----
            for m in range(M_TILES):
                if m + 6 < M_TILES:
                    load_x(m + 6)
                if m == 8:
                    load_w(2)
                do_group(0, m)

            # ---- passes 2..8: remaining columns, m-inner (w2 was
            # prefetched during pass 1, so prefetch j+2 here) ----
            for j in range(1, N_TILES):
                for m in range(M_TILES):
                    if m == 0 and j + 2 < N_TILES:
                        load_w(j + 2, bounds=[0, 16, 32])
                    do_group(j, m)

    nc.compile()
    _cached_nc = nc
    return nc


def make_in_maps(x, weight, bias):
    import ml_dtypes

    x = np.asarray(x)
    weight = np.asarray(weight)
    bias = np.ascontiguousarray(np.asarray(bias))
    assert x.dtype == np.float16 and weight.dtype == np.float16

    # host-side fp8 quantization (RNE, matches the reference's jnp casts:
    # |x| << e5m2 max, |w| << 240 so OCP e4m3fn bits == TRN e4m3 values)
    x8 = x.astype(ml_dtypes.float8_e5m2).view(np.uint8)
    w8 = weight.astype(ml_dtypes.float8_e4m3fn).view(np.uint8)

    # weight [DOUT, DIN] -> [j, ki, ko, n]: wd[j,ki,ko,n] = w8[j*512+n,
    # ko*128+ki] (i.e. weight.T in per-tile K-major blocks)
    wd = np.ascontiguousarray(
        w8.reshape(N_TILES, N_TILE, K_SUB, P).transpose(0, 3, 2, 1))

    xf = x8.reshape(M_TOTAL, DIN)
    in_maps = []
    for c in range(NCORES):
        xc = xf[c * M_LOC:(c + 1) * M_LOC]
        # [M_LOC, DIN] -> [m-tile, ki, ko, m]: xd[t,ki,ko,m] = xc[t*128+m,
        # ko*128+ki]
        xd = np.ascontiguousarray(
            xc.reshape(M_TILES, P, K_SUB, P).transpose(0, 3, 2, 1))
        in_maps.append({"xd": xd, "wd": wd, "bvec": bias})
    return in_maps


def gather_out(results):
    out = np.concatenate([r["out"] for r in results], axis=0)
    return out.reshape(B, S, DOUT)


def kernel(x, weight, bias):
    from concourse.bass_utils import run_bass_kernel_spmd

    nc = _build()
    in_maps = make_in_maps(x, weight, bias)
    res = run_bass_kernel_spmd(nc, in_maps, core_ids=list(range(NCORES)))
    return gather_out(res.results)


# revision 9
# speedup vs baseline: 1.0167x; 1.0071x over previous
"""FP8 GEMM kernel for Trainium2 (8 NeuronCores, SPMD data-parallel over tokens).

Computes: out = fp16( fp32( e5m2(x) @ e4m3(weight.T) ) + bias )
  x      [4, 4096, 4096] fp16
  weight [4096, 4096]    fp16  (out_features, in_features)
  bias   [4096]          fp16
  out    [4, 4096, 4096] fp16

Sharding: token dim (B*S = 16384) split across 8 cores (2048 rows each);
weight + bias replicated. No collectives; host concatenates the outputs.

Layout: the host quantizes to fp8 (ml_dtypes RNE — bit-identical to the
reference's jnp casts for these value ranges) and pre-packs both operands
into per-tile K-major blocks (`[tile][ki=128][ko=32][free]`), so every
device load is a fully contiguous plain-copy DMA. fp8 bits ship as uint8
tensors (the PJRT path rejects fp8 dtypes) and are bitcast at DMA issue.

Per-core kernel:
 - Loads are plain HWDGE copies (no cast): weights stream on the scalar
   (ACT) ring, x/bias/stores on the sync (SP) ring. Half the HBM bytes of
   the fp16+cast path and ~2x the bandwidth, so delivery stays ahead of
   the PE from ~1us on (the old SWDGE cast path idled the PE for 29us).
 - DoubleRow fp8 matmuls (K=256/instr, moving free dim 2x512 at the
   ~213ns/MM streaming floor) accumulate fp32 into PSUM. All of x8 stays
   resident (64KB/part); w8 n-tiles stream through a 3-deep pool.
 - The first two n-tile columns are interleaved per m-tile so the ramp
   only needs one x tile per ~6.9us of PE work.
 - Bias add fused into the PSUM eviction on DVE (its only job); output
   stores + bias broadcast go out on the sync HWDGE queue.
"""

import sys

if "/opt/trn_rl_repo" not in sys.path:
    sys.path.insert(0, "/opt/trn_rl_repo")

import numpy as np

B, S, DIN, DOUT = 4, 4096, 4096, 4096
NCORES = 8
M_TOTAL = B * S              # 16384
M_LOC = M_TOTAL // NCORES    # 2048
P = 128
M_TILES = M_LOC // P         # 16 m-tiles of 128 rows
N_TILE = 512
N_TILES = DOUT // N_TILE     # 8
K_SUB = DIN // P             # 32 k-subtiles of 128
K_CHUNKS = K_SUB // 2        # 16 DoubleRow chunks of 256

_cached_nc = None


def _build():
    global _cached_nc
    if _cached_nc is not None:
        return _cached_nc

    import concourse.mybir as mybir
    import concourse.tile as tile
    from concourse import bacc

    nc = bacc.Bacc("TRN2", target_bir_lowering=False, debug=False,
                   num_devices=NCORES)

    # host-packed fp8 K-major tile blocks, shipped as uint8 bits
    xd = nc.dram_tensor("xd", [M_TILES, P, K_SUB, P], mybir.dt.uint8,
                        kind="ExternalInput")
    wd = nc.dram_tensor("wd", [N_TILES, P, K_SUB, N_TILE], mybir.dt.uint8,
                        kind="ExternalInput")
    bvec = nc.dram_tensor("bvec", [DOUT], mybir.dt.float16,
                          kind="ExternalInput")
    out = nc.dram_tensor("out", [M_LOC, DOUT], mybir.dt.float16,
                         kind="ExternalOutput")

    with tile.TileContext(nc) as tc:
        with tc.tile_pool(name="w8p", bufs=3) as w8p, \
             tc.tile_pool(name="x8p", bufs=1) as x8p, \
             tc.tile_pool(name="outp", bufs=8) as outp, \
             tc.tile_pool(name="cst", bufs=1) as cst, \
             tc.tile_pool(name="psum", bufs=8, space="PSUM") as psump:

            # resident fp8 x: 16 tiles of [ki, ko, 128] e5m2
            x8 = [x8p.tile([P, K_SUB, P], mybir.dt.float8e5,
                           tag=f"x8_{m}", name=f"x8_{m}")
                  for m in range(M_TILES)]

            w8 = {}

            def load_w(j, bounds=None, eng=None):
                # bounds: ko-split points so the first matmuls can start
                # before the whole tile lands
                w8[j] = w8p.tile([P, K_SUB, N_TILE], mybir.dt.float8e4,
                                 tag="w8", name=f"w8_{j}")
                if bounds is None:
                    bounds = [0, K_SUB]
                if eng is None:
                    eng = nc.scalar
                for a, b in zip(bounds[:-1], bounds[1:]):
                    ko = slice(a, b)
                    eng.dma_start(
                        w8[j][:, ko, :],
                        wd[j, :, ko, :].bitcast(mybir.dt.float8e4))

            def load_x(m):
                nc.sync.dma_start(
                    x8[m][:],
                    xd[m, :, :, :].bitcast(mybir.dt.float8e5))

            # prologue: only x8[0] + w8[0] gate the first matmuls, so they
            # get both HWDGE rings to themselves (w0 chunks alternate
            # rings); everything else (x1-5, w1, bias) queues behind.
            # All-8-cores-simultaneous HBM pull runs ~250GB/s per core, so
            # w0 lands ~15us in and pass 1 (j=0 only) is PE-bound after.
            load_x(0)
            w8[0] = w8p.tile([P, K_SUB, N_TILE], mybir.dt.float8e4,
                             tag="w8", name="w8_0")
            w0_bounds = [0, 2, 8, 14, 20, 26, 32]
            for i, (a, b) in enumerate(zip(w0_bounds[:-1], w0_bounds[1:])):
                eng = nc.scalar if i % 2 == 0 else nc.sync
                eng.dma_start(w8[0][:, a:b, :],
                              wd[0, :, a:b, :].bitcast(mybir.dt.float8e4))
            # x tiles next — pass 1 consumes one every ~3.5us; bias before
            # w1 (first PSUM eviction needs it, and w1 isn't touched until
            # pass 2 at ~75us)
            for m in range(1, 6):
                load_x(m)
            bias_rep = cst.tile([P, DOUT], mybir.dt.float16)
            nc.sync.dma_start(bias_rep[:],
                              bvec.ap()[None, :].to_broadcast((P, DOUT)))
            load_w(1, bounds=[0, 8, 16, 24, 32])

            def do_group(j, m):
                wtile = w8[j]
                ps = psump.tile([P, N_TILE], mybir.dt.float32, tag="ps",
                                name=f"ps_{j}_{m}")
                for kc in range(K_CHUNKS):
                    nc.tensor.matmul(
                        ps[:],
                        x8[m][:, 2 * kc:2 * kc + 2, :],
                        wtile[:, 2 * kc:2 * kc + 2, :],
                        start=(kc == 0),
                        stop=(kc == K_CHUNKS - 1),
                        perf_mode=mybir.MatmulPerfMode.DoubleRow,
                    )
                ob = outp.tile([P, N_TILE], mybir.dt.float16, tag="ob",
                               name=f"ob_{j}_{m}")
                nc.vector.tensor_add(
                    ob[:], ps[:], bias_rep[:, j * N_TILE:(j + 1) * N_TILE])
                nc.sync.dma_start(
                    out[m * P:(m + 1) * P,
                        j * N_TILE:(j + 1) * N_TILE], ob[:])

            # ---- pass 1: column 0 alone — its only upfront needs are x0
            # and w0; x tiles + later w tiles stream in behind ----
            for m in range(M_TILES):
                if m + 6 < M_TILES:
                    load_x(m + 6)
                if m == 8:
                    load_w(2)
                do_group(0, m)

            # ---- passes 2..8: remaining columns, m-inner (w2 was
            # prefetched during pass 1, so prefetch j+2 here) ----
            for j in range(1, N_TILES):
                for m in range(M_TILES):
                    if m == 0 and j + 2 < N_TILES:
                        load_w(j + 2, bounds=[0, 16, 32])
                    do_group(j, m)

    nc.compile()
    _cached_nc = nc
    return nc


def make_in_maps(x, weight, bias):
    import ml_dtypes

    x = np.asarray(x)
    weight = np.asarray(weight)
    bias = np.ascontiguousarray(np.asarray(bias))
    assert x.dtype == np.float16 and weight.dtype == np.float16

    # host-side fp8 quantization (RNE, matches the reference's jnp casts:
    # |x| << e5m2 max, |w| << 240 so OCP e4m3fn bits == TRN e4m3 values)
    x8 = x.astype(ml_dtypes.float8_e5m2).view(np.uint8)
    w8 = weight.astype(ml_dtypes.float8_e4m3fn).view(np.uint8)

    # weight [DOUT, DIN] -> [j, ki, ko, n]: wd[j,ki,ko,n] = w8[j*512+n,
    # ko*128+ki] (i.e. weight.T in per-tile K-major blocks)
    wd = np.ascontiguousarray(
        w8.reshape(N_TILES, N_TILE, K_SUB, P).transpose(0, 3, 2, 1))

    xf = x8.reshape(M_TOTAL, DIN)
    in_maps = []
    for c in range(NCORES):
        xc = xf[c * M_LOC:(c + 1) * M_LOC]
        # [M_LOC, DIN] -> [m-tile, ki, ko, m]: xd[t,ki,ko,m] = xc[t*128+m,
        # ko*128+ki]
        xd = np.ascontiguousarray(
            xc.reshape(M_TILES, P, K_SUB, P).transpose(0, 3, 2, 1))
        in_maps.append({"xd": xd, "wd": wd, "bvec": bias})
    return in_maps


def gather_out(results):
    out = np.concatenate([r["out"] for r in results], axis=0)
    return out.reshape(B, S, DOUT)


def kernel(x, weight, bias):
    from concourse.bass_utils import run_bass_kernel_spmd

    nc = _build()
    in_maps = make_in_maps(x, weight, bias)
    res = run_bass_kernel_spmd(nc, in_maps, core_ids=list(range(NCORES)))
    return gather_out(res.results)


# revision 10
# speedup vs baseline: 1.0178x; 1.0011x over previous
"""FP8 GEMM kernel for Trainium2 (8 NeuronCores, SPMD data-parallel over tokens).

Computes: out = fp16( fp32( e5m2(x) @ e4m3(weight.T) ) + bias )
  x      [4, 4096, 4096] fp16
  weight [4096, 4096]    fp16  (out_features, in_features)
  bias   [4096]          fp16
  out    [4, 4096, 4096] fp16

Sharding: token dim (B*S = 16384) split across 8 cores (2048 rows each);
weight + bias replicated. No collectives; host concatenates the outputs.

Layout: the host quantizes to fp8 (ml_dtypes RNE — bit-identical to the
reference's jnp casts for these value ranges) and pre-packs both operands
into per-tile K-major blocks (`[tile][ki=128][ko=32][free]`), so every
device load is a fully contiguous plain-copy DMA. fp8 bits ship as uint8
tensors (the PJRT path rejects fp8 dtypes) and are bitcast at DMA issue.

Per-core kernel:
 - Loads are plain HWDGE copies (no cast): weights stream on the scalar
   (ACT) ring, x/bias/stores on the sync (SP) ring. Half the HBM bytes of
   the fp16+cast path and ~2x the bandwidth, so delivery stays ahead of
   the PE from ~1us on (the old SWDGE cast path idled the PE for 29us).
 - DoubleRow fp8 matmuls (K=256/instr, moving free dim 2x512 at the
   ~213ns/MM streaming floor) accumulate fp32 into PSUM. All of x8 stays
   resident (64KB/part); w8 n-tiles stream through a 3-deep pool.
 - The first two n-tile columns are interleaved per m-tile so the ramp
   only needs one x tile per ~6.9us of PE work.
 - Bias add fused into the PSUM eviction on DVE (its only job); output
   stores + bias broadcast go out on the sync HWDGE queue.
"""

import sys

if "/opt/trn_rl_repo" not in sys.path:
    sys.path.insert(0, "/opt/trn_rl_repo")

import numpy as np

B, S, DIN, DOUT = 4, 4096, 4096, 4096
NCORES = 8
M_TOTAL = B * S              # 16384
M_LOC = M_TOTAL // NCORES    # 2048
P = 128
M_TILES = M_LOC // P         # 16 m-tiles of 128 rows
N_TILE = 512
N_TILES = DOUT // N_TILE     # 8
K_SUB = DIN // P             # 32 k-subtiles of 128
K_CHUNKS = K_SUB // 2        # 16 DoubleRow chunks of 256

_cached_nc = None


def _build():
    global _cached_nc
    if _cached_nc is not None:
        return _cached_nc

    import concourse.mybir as mybir
    import concourse.tile as tile
    from concourse import bacc

    nc = bacc.Bacc("TRN2", target_bir_lowering=False, debug=False,
                   num_devices=NCORES)

    # host-packed fp8 K-major tile blocks, shipped as uint8 bits
    xd = nc.dram_tensor("xd", [M_TILES, P, K_SUB, P], mybir.dt.uint8,
                        kind="ExternalInput")
    wd = nc.dram_tensor("wd", [N_TILES, P, K_SUB, N_TILE], mybir.dt.uint8,
                        kind="ExternalInput")
    bvec = nc.dram_tensor("bvec", [DOUT], mybir.dt.float16,
                          kind="ExternalInput")
    out = nc.dram_tensor("out", [M_LOC, DOUT], mybir.dt.float16,
                         kind="ExternalOutput")

    with tile.TileContext(nc) as tc:
        with tc.tile_pool(name="w8p", bufs=3) as w8p, \
             tc.tile_pool(name="x8p", bufs=1) as x8p, \
             tc.tile_pool(name="outp", bufs=8) as outp, \
             tc.tile_pool(name="cst", bufs=1) as cst, \
             tc.tile_pool(name="psum", bufs=8, space="PSUM") as psump:

            # resident fp8 x: 16 tiles of [ki, ko, 128] e5m2
            x8 = [x8p.tile([P, K_SUB, P], mybir.dt.float8e5,
                           tag=f"x8_{m}", name=f"x8_{m}")
                  for m in range(M_TILES)]

            w8 = {}

            def load_w(j, bounds=None, eng=None):
                # bounds: ko-split points so the first matmuls can start
                # before the whole tile lands
                w8[j] = w8p.tile([P, K_SUB, N_TILE], mybir.dt.float8e4,
                                 tag="w8", name=f"w8_{j}")
                if bounds is None:
                    bounds = [0, K_SUB]
                if eng is None:
                    eng = nc.scalar
                for a, b in zip(bounds[:-1], bounds[1:]):
                    ko = slice(a, b)
                    eng.dma_start(
                        w8[j][:, ko, :],
                        wd[j, :, ko, :].bitcast(mybir.dt.float8e4))

            def load_x(m):
                nc.sync.dma_start(
                    x8[m][:],
                    xd[m, :, :, :].bitcast(mybir.dt.float8e5))

            # prologue: only x8[0] + w8[0] gate the first matmuls, so they
            # get both HWDGE rings to themselves, fine-chunked in exactly
            # the kc order the first psum group consumes; everything else
            # (x1-5, w1, bias) queues behind. All-8-cores-simultaneous HBM
            # pull runs ~250GB/s per core, so w0 lands ~17us in and pass 1
            # (j=0 only) is PE-bound after.
            w8[0] = w8p.tile([P, K_SUB, N_TILE], mybir.dt.float8e4,
                             tag="w8", name="w8_0")

            def w0_chunk(a, b, eng):
                eng.dma_start(w8[0][:, a:b, :],
                              wd[0, :, a:b, :].bitcast(mybir.dt.float8e4))

            def x_chunk(m, a, b):
                nc.sync.dma_start(
                    x8[m][:, a:b, :],
                    xd[m, :, a:b, :].bitcast(mybir.dt.float8e5))

            x_chunk(0, 0, 8)
            w0_chunk(0, 1, nc.scalar)
            w0_chunk(1, 2, nc.sync)
            w0_chunk(2, 4, nc.scalar)
            load_x(1)
            w0_chunk(4, 8, nc.sync)
            w0_chunk(8, 12, nc.scalar)
            x_chunk(0, 8, 32)
            w0_chunk(12, 18, nc.sync)
            w0_chunk(18, 24, nc.scalar)
            w0_chunk(24, 32, nc.sync)
            load_x(2)
            load_x(3)
            load_x(4)
            load_x(5)
            # bias before w1: the first PSUM eviction needs it, and w1
            # isn't touched until pass 2 at ~75us
            bias_rep = cst.tile([P, DOUT], mybir.dt.float16)
            nc.sync.dma_start(bias_rep[:],
                              bvec.ap()[None, :].to_broadcast((P, DOUT)))
            load_w(1, bounds=[0, 8, 16, 24, 32])

            def do_group(j, m):
                wtile = w8[j]
                ps = psump.tile([P, N_TILE], mybir.dt.float32, tag="ps",
                                name=f"ps_{j}_{m}")
                for kc in range(K_CHUNKS):
                    nc.tensor.matmul(
                        ps[:],
                        x8[m][:, 2 * kc:2 * kc + 2, :],
                        wtile[:, 2 * kc:2 * kc + 2, :],
                        start=(kc == 0),
                        stop=(kc == K_CHUNKS - 1),
                        perf_mode=mybir.MatmulPerfMode.DoubleRow,
                    )
                ob = outp.tile([P, N_TILE], mybir.dt.float16, tag="ob",
                               name=f"ob_{j}_{m}")
                nc.vector.tensor_add(
                    ob[:], ps[:], bias_rep[:, j * N_TILE:(j + 1) * N_TILE])
                nc.sync.dma_start(
                    out[m * P:(m + 1) * P,
                        j * N_TILE:(j + 1) * N_TILE], ob[:])

            # ---- pass 1: column 0 alone — its only upfront needs are x0
            # and w0; x tiles + later w tiles stream in behind ----
            for m in range(M_TILES):
                if m + 6 < M_TILES:
                    load_x(m + 6)
                if m == 8:
                    load_w(2)
                do_group(0, m)

            # ---- passes 2..8: remaining columns, m-inner (w2 was
            # prefetched during pass 1, so prefetch j+2 here) ----
            for j in range(1, N_TILES):
                for m in range(M_TILES):
                    if m == 0 and j + 2 < N_TILES:
                        load_w(j + 2, bounds=[0, 16, 32])
                    do_group(j, m)

    nc.compile()
    _cached_nc = nc
    return nc


def make_in_maps(x, weight, bias):
    import ml_dtypes

    x = np.asarray(x)
    weight = np.asarray(weight)
    bias = np.ascontiguousarray(np.asarray(bias))
    assert x.dtype == np.float16 and weight.dtype == np.float16

    # host-side fp8 quantization (RNE, matches the reference's jnp casts:
    # |x| << e5m2 max, |w| << 240 so OCP e4m3fn bits == TRN e4m3 values)
    x8 = x.astype(ml_dtypes.float8_e5m2).view(np.uint8)
    w8 = weight.astype(ml_dtypes.float8_e4m3fn).view(np.uint8)

    # weight [DOUT, DIN] -> [j, ki, ko, n]: wd[j,ki,ko,n] = w8[j*512+n,
    # ko*128+ki] (i.e. weight.T in per-tile K-major blocks)
    wd = np.ascontiguousarray(
        w8.reshape(N_TILES, N_TILE, K_SUB, P).transpose(0, 3, 2, 1))

    xf = x8.reshape(M_TOTAL, DIN)
    in_maps = []
    for c in range(NCORES):
        xc = xf[c * M_LOC:(c + 1) * M_LOC]
        # [M_LOC, DIN] -> [m-tile, ki, ko, m]: xd[t,ki,ko,m] = xc[t*128+m,
        # ko*128+ki]
        xd = np.ascontiguousarray(
            xc.reshape(M_TILES, P, K_SUB, P).transpose(0, 3, 2, 1))
        in_maps.append({"xd": xd, "wd": wd, "bvec": bias})
    return in_maps


def gather_out(results):
    out = np.concatenate([r["out"] for r in results], axis=0)
    return out.reshape(B, S, DOUT)


def kernel(x, weight, bias):
    from concourse.bass_utils import run_bass_kernel_spmd

    nc = _build()
    in_maps = make_in_maps(x, weight, bias)
    res = run_bass_kernel_spmd(nc, in_maps, core_ids=list(range(NCORES)))
    return gather_out(res.results)


# revision 12
# speedup vs baseline: 1.0265x; 1.0085x over previous
"""FP8 GEMM kernel for Trainium2 (8 NeuronCores, SPMD data-parallel over tokens).

Computes: out = fp16( fp32( e5m2(x) @ e4m3(weight.T) ) + bias )
  x      [4, 4096, 4096] fp16
  weight [4096, 4096]    fp16  (out_features, in_features)
  bias   [4096]          fp16
  out    [4, 4096, 4096] fp16

Sharding: token dim (B*S = 16384) split across 8 cores (2048 rows each);
weight + bias replicated. No collectives; host concatenates the outputs.

Layout: the host quantizes to fp8 (ml_dtypes RNE — bit-identical to the
reference's jnp casts for these value ranges) and pre-packs both operands
into per-tile K-major blocks (`[tile][ki=128][ko=32][free]`), so every
device load is a fully contiguous plain-copy DMA. fp8 bits ship as uint8
tensors (the PJRT path rejects fp8 dtypes) and are bitcast at DMA issue.

Per-core kernel:
 - Loads are plain HWDGE copies (no cast): weights stream on the scalar
   (ACT) ring, x/bias/stores on the sync (SP) ring. Half the HBM bytes of
   the fp16+cast path and ~2x the bandwidth, so delivery stays ahead of
   the PE from ~1us on (the old SWDGE cast path idled the PE for 29us).
 - DoubleRow fp8 matmuls (K=256/instr, moving free dim 2x512 at the
   ~213ns/MM streaming floor) accumulate fp32 into PSUM. All of x8 stays
   resident (64KB/part); w8 n-tiles stream through a 3-deep pool.
 - The first two n-tile columns are interleaved per m-tile so the ramp
   only needs one x tile per ~6.9us of PE work.
 - Bias add fused into the PSUM eviction on DVE (its only job); output
   stores + bias broadcast go out on the sync HWDGE queue.
"""

import sys

if "/opt/trn_rl_repo" not in sys.path:
    sys.path.insert(0, "/opt/trn_rl_repo")

import numpy as np

B, S, DIN, DOUT = 4, 4096, 4096, 4096
NCORES = 8
M_TOTAL = B * S              # 16384
M_LOC = M_TOTAL // NCORES    # 2048
P = 128
M_TILES = M_LOC // P         # 16 m-tiles of 128 rows
N_TILE = 512
N_TILES = DOUT // N_TILE     # 8
K_SUB = DIN // P             # 32 k-subtiles of 128
K_CHUNKS = K_SUB // 2        # 16 DoubleRow chunks of 256

_cached_nc = None


def _build():
    global _cached_nc
    if _cached_nc is not None:
        return _cached_nc

    import concourse.mybir as mybir
    import concourse.tile as tile
    from concourse import bacc

    nc = bacc.Bacc("TRN2", target_bir_lowering=False, debug=False,
                   num_devices=NCORES)

    # host-packed fp8 K-major tile blocks, shipped as uint8 bits
    xd = nc.dram_tensor("xd", [M_TILES, P, K_SUB, P], mybir.dt.uint8,
                        kind="ExternalInput")
    wd = nc.dram_tensor("wd", [N_TILES, P, K_SUB, N_TILE], mybir.dt.uint8,
                        kind="ExternalInput")
    bvec = nc.dram_tensor("bvec", [DOUT], mybir.dt.float16,
                          kind="ExternalInput")
    out = nc.dram_tensor("out", [M_LOC, DOUT], mybir.dt.float16,
                         kind="ExternalOutput")

    with tile.TileContext(nc) as tc:
        with tc.tile_pool(name="w8p", bufs=3) as w8p, \
             tc.tile_pool(name="x8p", bufs=1) as x8p, \
             tc.tile_pool(name="outp", bufs=8) as outp, \
             tc.tile_pool(name="cst", bufs=1) as cst, \
             tc.tile_pool(name="psum", bufs=8, space="PSUM") as psump:

            # resident fp8 x: 16 tiles of [ki, ko, 128] e5m2
            x8 = [x8p.tile([P, K_SUB, P], mybir.dt.float8e5,
                           tag=f"x8_{m}", name=f"x8_{m}")
                  for m in range(M_TILES)]

            w8 = {}

            def load_w(j, bounds=None, eng=None):
                # bounds: ko-split points so the first matmuls can start
                # before the whole tile lands
                w8[j] = w8p.tile([P, K_SUB, N_TILE], mybir.dt.float8e4,
                                 tag="w8", name=f"w8_{j}")
                if bounds is None:
                    bounds = [0, K_SUB]
                if eng is None:
                    eng = nc.scalar
                for a, b in zip(bounds[:-1], bounds[1:]):
                    ko = slice(a, b)
                    eng.dma_start(
                        w8[j][:, ko, :],
                        wd[j, :, ko, :].bitcast(mybir.dt.float8e4))

            def load_x(m):
                nc.sync.dma_start(
                    x8[m][:],
                    xd[m, :, :, :].bitcast(mybir.dt.float8e5))

            # prologue: only x8[0] + w8[0] gate the first matmuls, so they
            # get both HWDGE rings to themselves, fine-chunked in exactly
            # the kc order the first psum group consumes; everything else
            # (x1-5, w1, bias) queues behind. All-8-cores-simultaneous HBM
            # pull runs ~250GB/s per core, so w0 lands ~17us in and pass 1
            # (j=0 only) is PE-bound after.
            w8[0] = w8p.tile([P, K_SUB, N_TILE], mybir.dt.float8e4,
                             tag="w8", name="w8_0")

            def w0_chunk(a, b, eng):
                eng.dma_start(w8[0][:, a:b, :],
                              wd[0, :, a:b, :].bitcast(mybir.dt.float8e4))

            def x_chunk(m, a, b):
                nc.sync.dma_start(
                    x8[m][:, a:b, :],
                    xd[m, :, a:b, :].bitcast(mybir.dt.float8e5))

            x_chunk(0, 0, 8)
            w0_chunk(0, 1, nc.scalar)
            w0_chunk(1, 2, nc.sync)
            w0_chunk(2, 4, nc.scalar)
            load_x(1)
            w0_chunk(4, 8, nc.sync)
            w0_chunk(8, 12, nc.scalar)
            x_chunk(0, 8, 32)
            w0_chunk(12, 18, nc.sync)
            w0_chunk(18, 24, nc.scalar)
            w0_chunk(24, 32, nc.sync)
            for m in range(2, 8):
                load_x(m)
            # bias before the remaining x tiles: the first PSUM eviction
            # (~18us) needs it
            bias_rep = cst.tile([P, DOUT], mybir.dt.float16)
            nc.sync.dma_start(bias_rep[:],
                              bvec.ap()[None, :].to_broadcast((P, DOUT)))
            for m in range(8, M_TILES):
                load_x(m)
            # w1 rides the sync ring too, FIFO behind every x tile, so it
            # cannot steal HBM bandwidth during the 8-core ramp rush; it
            # still lands well before pass 2 (~75us)
            load_w(1, bounds=[0, 8, 16, 24, 32], eng=nc.sync)

            def do_group(j, m):
                wtile = w8[j]
                ps = psump.tile([P, N_TILE], mybir.dt.float32, tag="ps",
                                name=f"ps_{j}_{m}")
                for kc in range(K_CHUNKS):
                    nc.tensor.matmul(
                        ps[:],
                        x8[m][:, 2 * kc:2 * kc + 2, :],
                        wtile[:, 2 * kc:2 * kc + 2, :],
                        start=(kc == 0),
                        stop=(kc == K_CHUNKS - 1),
                        perf_mode=mybir.MatmulPerfMode.DoubleRow,
                    )
                ob = outp.tile([P, N_TILE], mybir.dt.float16, tag="ob",
                               name=f"ob_{j}_{m}")
                nc.vector.tensor_add(
                    ob[:], ps[:], bias_rep[:, j * N_TILE:(j + 1) * N_TILE])
                nc.sync.dma_start(
                    out[m * P:(m + 1) * P,
                        j * N_TILE:(j + 1) * N_TILE], ob[:])

            # ---- pass 1: column 0 alone — its only upfront needs are x0
            # and w0; x tiles + later w tiles stream in behind ----
            for m in range(M_TILES):
                if m == 8:
                    load_w(2)
                do_group(0, m)

            # ---- passes 2..8: remaining columns, m-inner (w2 was
            # prefetched during pass 1, so prefetch j+2 here) ----
            for j in range(1, N_TILES):
                for m in range(M_TILES):
                    if m == 0 and j + 2 < N_TILES:
                        load_w(j + 2, bounds=[0, 16, 32])
                    do_group(j, m)

    nc.compile()
    _cached_nc = nc
    return nc


def make_in_maps(x, weight, bias):
    import ml_dtypes

    x = np.asarray(x)
    weight = np.asarray(weight)
    bias = np.ascontiguousarray(np.asarray(bias))
    assert x.dtype == np.float16 and weight.dtype == np.float16

    # host-side fp8 quantization (RNE, matches the reference's jnp casts:
    # |x| << e5m2 max, |w| << 240 so OCP e4m3fn bits == TRN e4m3 values)
    x8 = x.astype(ml_dtypes.float8_e5m2).view(np.uint8)
    w8 = weight.astype(ml_dtypes.float8_e4m3fn).view(np.uint8)

    # weight [DOUT, DIN] -> [j, ki, ko, n]: wd[j,ki,ko,n] = w8[j*512+n,
    # ko*128+ki] (i.e. weight.T in per-tile K-major blocks)
    wd = np.ascontiguousarray(
        w8.reshape(N_TILES, N_TILE, K_SUB, P).transpose(0, 3, 2, 1))

    xf = x8.reshape(M_TOTAL, DIN)
    in_maps = []
    for c in range(NCORES):
        xc = xf[c * M_LOC:(c + 1) * M_LOC]
        # [M_LOC, DIN] -> [m-tile, ki, ko, m]: xd[t,ki,ko,m] = xc[t*128+m,
        # ko*128+ki]
        xd = np.ascontiguousarray(
            xc.reshape(M_TILES, P, K_SUB, P).transpose(0, 3, 2, 1))
        in_maps.append({"xd": xd, "wd": wd, "bvec": bias})
    return in_maps


def gather_out(results):
    out = np.concatenate([r["out"] for r in results], axis=0)
    return out.reshape(B, S, DOUT)


def kernel(x, weight, bias):
    from concourse.bass_utils import run_bass_kernel_spmd

    nc = _build()
    in_maps = make_in_maps(x, weight, bias)
    res = run_bass_kernel_spmd(nc, in_maps, core_ids=list(range(NCORES)))
    return gather_out(res.results)
